# revision 3
# baseline (speedup 1.0000x reference)
"""Trainium2 Bass kernel for the Jacobian-log-det-squared loss.

Reference computation (per voxel of a (B,C=3,D,H,W) displacement field):
    J[j,i] = d(u_i)/d(x_j) + delta_ij   (numpy-style gradient: central in the
             interior, one-sided at the boundary), out = log(det(J))**2.

Strategy
--------
* Shard over (B=2) x (H quarters=4) -> 8 cores, pure data parallel. Each core
  receives a halo-padded slab (3, 128, 50, 194) and computes (128, 48, 192).
* Host pads ghost cells (2*x0 - x1) along H and W so one-sided boundary
  differences become central differences of the padded array - the device
  kernel is a uniform interior stencil with no edge special-casing.
* D axis (=128) sits on SBUF partitions; the D-gradient (including its
  boundary rows) is one banded 128x128 matmul on the TensorEngine.
* H/W gradients are shifted-AP subtracts on the Vector engine (fp32).
* det(I+G) - 1 = c1 + c2 + c3 is evaluated with the dominant trace term (c1)
  in fp32 and the small quadratic/cubic terms in fp16 (DVE 2x mode). All
  gradient values are carried as 2*G (the banded matrix is scaled by 2, raw
  differences skip the 0.5) and the scale is folded into the final
  ScalarEngine activation: log(0.5 * zf + 1).
"""

import sys

sys.path.insert(0, "/opt/trn_rl_repo")

import numpy as np
from contextlib import ExitStack

import concourse.bass as bass  # noqa: F401  (registers engine types)
import concourse.tile as tile
from concourse import bacc, mybir
from concourse.bass_utils import run_bass_kernel_spmd
from concourse.bass_interp import get_hw_module

f32 = mybir.dt.float32
f16 = mybir.dt.float16
Act = mybir.ActivationFunctionType

B, C, D, H, W = 2, 3, 128, 192, 192
NCORES = 8
HQ = 4                 # H quarters (cores = B * HQ)
HL = H // HQ           # 48 output H rows per core
HCH = 8                # output H rows per chunk
NCH = HL // HCH        # 6 chunks
WP = W + 2             # W padded with ghost columns
HIN = HCH + 2          # input rows per chunk (halo)

_CACHE = {}


def _dmat2() -> np.ndarray:
    """Banded difference matrix (scaled by 2): out[m] = sum_k dmat[k,m] x[k]
    equals 2 * (numpy-gradient of x along D at m)."""
    m = np.zeros((D, D), np.float32)
    for j in range(1, D - 1):
        m[j - 1, j] = -1.0
        m[j + 1, j] = 1.0
    m[0, 0], m[1, 0] = -2.0, 2.0
    m[D - 2, D - 1], m[D - 1, D - 1] = -2.0, 2.0
    return m


def _build_program():
    nc = bacc.Bacc("TRN2", target_bir_lowering=False, debug=False,
                   num_devices=NCORES)
    x_in = nc.dram_tensor("x", [C, D, HL + 2, WP], f32,
                          kind="ExternalInput").ap()
    dm_in = nc.dram_tensor("dmat", [D, D], f32, kind="ExternalInput").ap()
    out_t = nc.dram_tensor("out", [D, HL, W], f32, kind="ExternalOutput").ap()

    FD = HCH * W  # 1536 free elements per chunk

    with tile.TileContext(nc) as tc, ExitStack() as ctx:
        const_pool = ctx.enter_context(tc.tile_pool(name="const", bufs=1))
        xin_pool = ctx.enter_context(tc.tile_pool(name="xin", bufs=2))
        ps_pool = ctx.enter_context(
            tc.tile_pool(name="ps", bufs=6, space="PSUM"))
        r32_pool = ctx.enter_context(tc.tile_pool(name="r32", bufs=2))
        h16_pool = ctx.enter_context(tc.tile_pool(name="h16", bufs=2))
        tmp_pool = ctx.enter_context(tc.tile_pool(name="t16", bufs=10))

        dmat = const_pool.tile([D, D], f32)
        nc.sync.dma_start(dmat[:], dm_in[:])

        for chi in range(NCH):
            h0 = chi * HCH  # top input row of this chunk (padded coords)

            # ---- load the three channel chunks --------------------------
            xv = []
            for c in range(C):
                t = xin_pool.tile([D, HIN * WP], f32, tag=f"x{c}", name=f"xin{c}")
                v = t[:].rearrange("p (h w) -> p h w", h=HIN, w=WP)
                nc.sync.dma_start(v, x_in[c, :, h0:h0 + HIN, :])
                xv.append(v)

            def hdiff(c):
                return (xv[c][:, 2:HIN, 1:WP - 1],
                        xv[c][:, 0:HIN - 2, 1:WP - 1])

            def wdiff(c):
                return (xv[c][:, 1:HIN - 1, 2:WP],
                        xv[c][:, 1:HIN - 1, 0:WP - 2])

            def t32(tag):
                return r32_pool.tile([D, FD], f32, tag=tag, name="r32_" + tag)

            def t16(tag):
                return h16_pool.tile([D, FD], f16, tag=tag, name="h16_" + tag)

            def tmp():
                return tmp_pool.tile([D, FD], f16, tag="tmp", name="tmp16")

            def v3(t):
                return t[:].rearrange("p (h w) -> p h w", h=HCH, w=W)

            # ---- raw differences (values are 2*G) -----------------------
            # fp32 diagonal-path diffs
            e_r = t32("e")   # 2*G11 = H-diff of u1
            nc.vector.tensor_sub(v3(e_r), *hdiff(1))
            i_r = t32("i")   # 2*G22 = W-diff of u2
            nc.vector.tensor_sub(v3(i_r), *wdiff(2))
            # fp16 off-diagonal diffs
            d16 = t16("d")   # 2*G10
            nc.vector.tensor_sub(v3(d16), *hdiff(0))
            fd16 = t16("f")  # 2*G12
            nc.vector.tensor_sub(v3(fd16), *hdiff(2))
            g16 = t16("g")   # 2*G20
            nc.vector.tensor_sub(v3(g16), *wdiff(0))
            hd16 = t16("h")  # 2*G21
            nc.vector.tensor_sub(v3(hd16), *wdiff(1))

            # fp16 copies of the diagonal diffs
            e16 = t16("e16")
            nc.vector.tensor_copy(e16[:], e_r[:])
            i16 = t16("i16")
            nc.vector.tensor_copy(i16[:], i_r[:])

            # T = 2*(G11 + G22)
            T2 = t32("T")
            nc.vector.tensor_add(T2[:], e_r[:], i_r[:])
            T16 = t16("T16")
            nc.vector.tensor_copy(T16[:], T2[:])

            # ---- TensorEngine: D-gradient rows (a,b,c = 2*G00,2*G01,2*G02)
            # and the fp32 c1 path zc = 2*(G00+G11+G22)
            zc2 = t32("z")
            ab16 = [t16(t) for t in ("a16", "b16", "c16")]
            for c in range(C):
                for r in range(HCH):
                    ps = ps_pool.tile([D, W], f32, tag="ps", name="ps")
                    nc.tensor.matmul(ps[:], dmat[:],
                                     xv[c][:, r + 1, 1:WP - 1])
                    sl = slice(r * W, (r + 1) * W)
                    nc.scalar.copy(ab16[c][:, sl], ps[:])
                    if c == 0:
                        nc.vector.tensor_add(zc2[:, sl], ps[:], T2[:, sl])
            a16, b16, c16 = ab16

            # ---- fp16 product chain -------------------------------------
            def mul(o, x, y):
                nc.vector.tensor_mul(o[:], x[:], y[:])

            def sub(o, x, y):
                nc.vector.tensor_sub(o[:], x[:], y[:])

            def add(o, x, y):
                nc.vector.tensor_add(o[:], x[:], y[:])

            pa, pb = tmp(), tmp()
            mul(pa, e16, i16)
            mul(pb, fd16, hd16)
            N0 = tmp()
            sub(N0, pa, pb)            # 4*(ei - fh)

            pa, pb = tmp(), tmp()
            mul(pa, d16, i16)
            mul(pb, fd16, g16)
            N1 = tmp()
            sub(N1, pa, pb)            # 4*(di - fg)
            Q2 = tmp()
            mul(Q2, b16, N1)           # 8*b(di - fg)

            pa, pb = tmp(), tmp()
            mul(pa, d16, hd16)
            mul(pb, e16, g16)
            N2 = tmp()
            sub(N2, pa, pb)            # 4*(dh - eg)
            Q3 = tmp()
            mul(Q3, c16, N2)           # 8*c(dh - eg)

            Q1 = tmp()
            mul(Q1, a16, N0)           # 8*a(ei - fh)
            Q4 = tmp()
            sub(Q4, Q1, Q2)
            R3 = tmp()
            add(R3, Q4, Q3)            # 8*c3

            U1 = tmp()
            mul(U1, a16, T16)          # 4*a(e+i)
            U2 = tmp()
            mul(U2, b16, d16)          # 4*bd
            U3 = tmp()
            mul(U3, c16, g16)          # 4*cg
            U4 = tmp()
            add(U4, U2, U3)
            U5 = tmp()
            sub(U5, U1, U4)            # 4*c2 - N0tilde

            V1 = tmp()
            add(V1, U5, N0)            # 4*c2
            R3h = tmp()
            nc.vector.tensor_scalar_mul(R3h[:], R3[:], 0.5)  # 4*c3
            V2 = tmp()
            add(V2, V1, R3h)           # 4*(c2+c3)

            V3 = t32("e")  # e_r is dead by now; share its slots
            nc.vector.tensor_scalar_mul(V3[:], V2[:], 0.5)   # 2*(c2+c3)

            # ---- final combine + log^2 ----------------------------------
            zf2 = t32("T")  # reuse T slot pool
            nc.vector.tensor_add(zf2[:], zc2[:], V3[:])      # 2*(det-1)
            lg = t32("i")  # i_r is dead; share
            nc.scalar.activation(lg[:], zf2[:], Act.Ln, bias=1.0, scale=0.5)
            ov = t32("z")  # zc2 is dead; share
            nc.scalar.activation(ov[:], lg[:], Act.Square)

            nc.sync.dma_start(
                out_t[:, chi * HCH:(chi + 1) * HCH, :], v3(ov))

    nc.compile()
    nc.m = get_hw_module(nc.m)
    return nc


def _get_program():
    if "nc" not in _CACHE:
        _CACHE["nc"] = _build_program()
    return _CACHE["nc"]


def kernel(x: np.ndarray) -> np.ndarray:
    x = np.asarray(x, dtype=np.float32)
    assert x.shape == (B, C, D, H, W)

    # ghost cells: 2*x[edge] - x[edge+1] makes the central difference of the
    # padded array equal the one-sided boundary difference of the original
    xw = np.concatenate(
        [2.0 * x[..., :1] - x[..., 1:2], x,
         2.0 * x[..., -1:] - x[..., -2:-1]], axis=-1)            # (B,C,D,H,194)
    xh = np.concatenate(
        [2.0 * xw[:, :, :, :1] - xw[:, :, :, 1:2], xw,
         2.0 * xw[:, :, :, -1:] - xw[:, :, :, -2:-1]], axis=3)   # (B,C,D,194,194)

    dmat = _dmat2()
    in_maps = []
    for core in range(NCORES):
        b, hq = divmod(core, HQ)
        slab = np.ascontiguousarray(xh[b, :, :, hq * HL: hq * HL + HL + 2, :])
        in_maps.append({"x": slab, "dmat": dmat})

    nc = _get_program()
    res = run_bass_kernel_spmd(nc, in_maps, list(range(NCORES)))

    out = np.empty((B, D, H, W), np.float32)
    for core in range(NCORES):
        b, hq = divmod(core, HQ)
        out[b, :, hq * HL:(hq + 1) * HL, :] = res.results[core]["out"]
    return out


if __name__ == "__main__":
    rng = np.random.default_rng(0)
    xt = (rng.standard_normal((B, C, D, H, W)) * 0.05).astype(np.float32)
    y = kernel(xt)
    print("out", y.shape, y.dtype, float(y.mean()))
